# revision 39
# baseline (speedup 1.0000x reference)
"""Trainium2 Bass kernel for windowed 3D cross-attention (CrossAttention3D).

Reference computation:
  (B=1, C=128, D=H=W=48) q/k/v maps -> channels-last -> partition into
  6x6x6 windows (512 windows, 216 tokens each) -> LayerNorm over C ->
  8-head attention within each window (dh=16) -> output projection ->
  window reverse -> (B, C, D, H, W).

Sharding: data-parallel over the window depth axis. Core m processes the
D-slab d in [6m, 6m+6) -- 64 independent windows per core. The host
rewrites each slab window-major ([C, 8, 8, 216]) so every access in the
kernel is contiguous, and un-permutes the output.

Kernel structure: a flat 4-stage software pipeline over the 64 windows,
one iteration per window index W:
  A: LN stats for window W      (x^2 on DVE/ACT + 12 tiny K=128 N=1
                                 PE matmuls; batched derived-stat math
                                 per eighth)
  B: normalization for W-8      (PE-broadcast stat rows; q^/k^ fp16 in
                                 channel-major; v^ PE-transposed to
                                 token-major with a ones column; q^/k^
                                 DMA-restaged head-major so all score
                                 matmuls read partition-base-0 operands)
  C: scores + exp for W-16      (16 fp16 matmuls; exp on ACT from PSUM)
  D: attn@v + softmax-divide + projection for W-17, interleaved with
     C's score groups so the PE never waits on exp.

All heavy matmuls use fp16 operands (1 PE cycle/row vs 4 for fp32).
gamma_kv/beta_kv fold into the projection weights/bias on the host
(softmax rows sum to 1); gamma_q/beta_q (if nontrivial) are one
per-partition tensor_scalar on q^.

Hardware constraints baked in (probed on device): PE operands must sit
at partition base 0/32/64 (explicit off-base tile_position faults);
mixing tile row-positions within one PSUM bank faults; matmul RHS APs
must be single-free-dim; GPSIMD cannot touch PSUM; TT cannot read two
PSUM operands; partition-strided DMA gathers silently read the wrong
partitions (only contiguous partition folds work).
"""
import sys

sys.path.insert(0, "/opt/trn_rl_repo")

from contextlib import ExitStack

import numpy as np

import concourse.bass as bass
import concourse.tile as tile
from concourse import bacc, mybir
from concourse.bass_utils import run_bass_kernel_spmd
from concourse.masks import make_identity

F32 = mybir.dt.float32
F16 = mybir.dt.float16
I32 = mybir.dt.int32
C = 128          # channels
NH = 8           # heads
DH = 16          # head dim
T = 216          # tokens per window (6*6*6)
TC = 108         # tokens per chunk (3 d-slices)
NCORES = 8
EPS = 1e-5
NW = 64          # windows per core

_BUILD_CACHE = {}


def _build_nc(trivial_q: bool, trivial_bias: bool, DEBUG=False):
    key = (trivial_q, trivial_bias, DEBUG)
    if key in _BUILD_CACHE:
        return _BUILD_CACHE[key]

    nc = bacc.Bacc("TRN2", target_bir_lowering=False, debug=False,
                   num_devices=NCORES)
    qs = nc.dram_tensor("q_slab", [C, 8, 8, T], F32, kind="ExternalInput")
    ks = nc.dram_tensor("k_slab", [C, 8, 8, T], F32, kind="ExternalInput")
    vs = nc.dram_tensor("v_slab", [C, 8, 8, T], F32, kind="ExternalInput")
    wt0 = nc.dram_tensor("wt0", [C, C], F16, kind="ExternalInput")
    wt1 = nc.dram_tensor("wt1", [C, C], F16, kind="ExternalInput")
    pb = nc.dram_tensor("pbias", [C, 1], F32, kind="ExternalInput")
    gq = bq = None
    if not trivial_q:
        gq = nc.dram_tensor("gq", [C, 1], F32, kind="ExternalInput")
        bq = nc.dram_tensor("bq", [C, 1], F32, kind="ExternalInput")
    ys = nc.dram_tensor("y_slab", [C, 8, 8, T], F32, kind="ExternalOutput")

    AF = mybir.ActivationFunctionType
    OP = mybir.AluOpType

    with tile.TileContext(nc) as tc, ExitStack() as ctx:
        consts = ctx.enter_context(tc.tile_pool(name="consts", bufs=1))
        inp = ctx.enter_context(tc.tile_pool(name="inp", bufs=3))
        outp = ctx.enter_context(tc.tile_pool(name="outp", bufs=2))
        sqp = ctx.enter_context(tc.tile_pool(name="sqp", bufs=2))
        xp = ctx.enter_context(tc.tile_pool(name="xp", bufs=2))
        qkp = ctx.enter_context(tc.tile_pool(name="qkp", bufs=2))
        qkh = ctx.enter_context(tc.tile_pool(name="qkh", bufs=9))
        tmpp = ctx.enter_context(tc.tile_pool(name="tmpp", bufs=2))
        sS = ctx.enter_context(tc.tile_pool(name="sS", bufs=2))
        ep = ctx.enter_context(tc.tile_pool(name="ep", bufs=3))
        avn_p = ctx.enter_context(tc.tile_pool(name="avn_p", bufs=2))
        rp = ctx.enter_context(tc.tile_pool(name="rp", bufs=2))
        # PSUM (8 banks x 2KB):
        # st 1 + scr 1 + bcn 2 + bp 1 + big(sc,y) 2 + av 1 = 8
        p_st = ctx.enter_context(tc.tile_pool(name="p_st", bufs=1, space="PSUM"))
        p_scr = ctx.enter_context(tc.tile_pool(name="p_scr", bufs=1, space="PSUM"))
        p_bcn = ctx.enter_context(tc.tile_pool(name="p_bcn", bufs=1, space="PSUM"))
        p_bp = ctx.enter_context(tc.tile_pool(name="p_bp", bufs=1, space="PSUM"))
        p_big = ctx.enter_context(tc.tile_pool(name="p_big", bufs=1, space="PSUM"))
        p_av = ctx.enter_context(tc.tile_pool(name="p_av", bufs=1, space="PSUM"))

        ident = consts.tile([C, C], F32)
        make_identity(nc, ident[:])
        ident16 = consts.tile([C, C], F16)
        nc.vector.tensor_copy(ident16[:], ident[:])
        ones_col = consts.tile([C, 1], F32)
        nc.vector.memset(ones_col[:], 1.0)
        ones_col16 = consts.tile([C, 1], F16)
        nc.vector.memset(ones_col16[:], 1.0)
        onesr16 = consts.tile([1, C], F16)
        nc.vector.memset(onesr16[:], 1.0)
        t_wt = []
        for hh, w_dram in enumerate((wt0, wt1)):
            t_w = consts.tile([C, C], F16, tag=f"wt{hh}")
            nc.sync.dma_start(t_w[:], w_dram[:, :])
            t_wt.append(t_w)
        t_pb = None
        if not trivial_bias:
            t_pb = consts.tile([C, 1], F32)
            nc.sync.dma_start(t_pb[:], pb[:, :])
        t_gq = t_bq = None
        if not trivial_q:
            t_gq = consts.tile([C, 1], F32)
            t_bq = consts.tile([C, 1], F32)
            nc.sync.dma_start(t_gq[:], gq[:, :])
            nc.sync.dma_start(t_bq[:], bq[:, :])
        # Derived LN stats (fp16), double-buffered per eighth.
        # Layout [TC, win, (rq, wq, rk, wk), jc]: the (stat, jc) free dims
        # are contiguous so one transpose yields rows 2*s + jc with each
        # tensor's 4 rows adjacent.
        t_D = [consts.tile([TC, 8, 4, 2], F16, tag=f"D{i}", name=f"D{i}")
               for i in range(2)]
        t_Dv = [consts.tile([TC, 2, 8, 2], F32, tag=f"Dv{i}", name=f"Dv{i}")
                for i in range(2)]
        # v^ stationary tiles (ring of 10): [chunk, hh, g, 32];
        # col 0 = ones (Z row), cols 1..16 = channels of head 4*hh+g.
        NTV = 10
        t_vh = []
        for w in range(NTV):
            tv = consts.tile([TC, 2, 2, 4, 32], F16, tag=f"tv{w}", name=f"tv{w}")
            nc.vector.memset(tv[:], 0.0)
            nc.vector.memset(tv[:, :, :, :, 0:1], 1.0)
            t_vh.append(tv)

        def rsqrt_cols(var_view, out_view, scale):
            """out = scale/sqrt(var); bit-trick seed + 2 Newton (no ACT
            table, keeps the activation table pinned on Exp)."""
            p = TC
            ve = sS.tile([p, 2, 8, 3], F32, tag="rs_ve")
            nc.vector.tensor_copy(ve[:], var_view)
            ti = sS.tile([p, 2, 8, 3], I32, tag="rs_ti")
            nc.vector.tensor_scalar(
                ti[:], ve[:].bitcast(I32), 1, None,
                op0=OP.logical_shift_right)
            nc.vector.tensor_scalar(
                ti[:], ti[:], 0, None, op0=OP.bitwise_not)
            nc.vector.tensor_scalar(
                ti[:], ti[:], 0x5f3759df + 1, None, op0=OP.add)
            y_cur = ti[:].bitcast(F32)
            t1 = sS.tile([p, 2, 8, 3], F32, tag="rs_t1")
            for it in range(2):
                nc.vector.tensor_tensor(t1[:], y_cur, y_cur, op=OP.mult)
                nc.vector.tensor_tensor(t1[:], t1[:], ve[:], op=OP.mult)
                nc.vector.tensor_scalar(t1[:], t1[:], -0.5, 1.5,
                                        op0=OP.mult, op1=OP.add)
                if it == 0:
                    yn = sS.tile([p, 2, 8, 3], F32, tag="rs_yn")
                    nc.vector.tensor_tensor(yn[:], t1[:], y_cur, op=OP.mult)
                    y_cur = yn[:]
                else:
                    nc.vector.tensor_tensor(t1[:], t1[:], y_cur, op=OP.mult)
                    nc.vector.tensor_scalar(out_view, t1[:], scale, None,
                                            op0=OP.mult)

        slabs = {}       # eighth -> (t_q, t_k, t_v)
        st_t = {}        # eighth -> stats PSUM tile
        oe_t = {}        # eighth -> output slab tile
        qkH_t = [None] * NW
        E_t = [None] * NW

        def load_slab(e):
            t_q = inp.tile([C, 8, T], F32, tag="in_q", name="t_q")
            t_k = inp.tile([C, 8, T], F32, tag="in_k", name="t_k")
            t_v = inp.tile([C, 8, T], F32, tag="in_v", name="t_v")
            nc.sync.dma_start(t_q[:], qs[:, e, :, :])
            nc.sync.dma_start(t_k[:], ks[:, e, :, :])
            nc.sync.dma_start(t_v[:], vs[:, e, :, :])
            slabs[e] = (t_q, t_k, t_v)

        load_slab(0)

        def stage_A(W):
            e, w = W // 8, W % 8
            if w == 0:
                if e + 1 < 8:
                    load_slab(e + 1)
                st_t[e] = p_st.tile([TC, 2, 8, 6], F32, tag="st", name="st")
            st = st_t[e]
            t_q, t_k, t_v = slabs[e]
            # st cols: (q_s1, q_s2, k_s1, k_s2, v_s1, v_s2)
            for it, srcT in enumerate((t_q, t_k, t_v)):
                win = srcT[:, w, :]
                # fp16 staging: fp16 stationary operands halve LDWEIGHTS
                # cost, and the fp16 square hits the DVE 2-byte fast path
                # (0.05% element error, irrelevant vs tolerance)
                x16 = xp.tile([C, T], F16, tag=f"x16{it}", name="x16")
                if it == 2:
                    nc.scalar.copy(x16[:], win)
                else:
                    nc.vector.tensor_copy(x16[:], win)
                sq = sqp.tile([C, T], F16, tag=f"sq{it}", name="sq")
                nc.vector.tensor_tensor(sq[:], x16[:], x16[:], op=OP.mult)
                for jc in range(2):
                    nc.tensor.matmul(st[:, jc, w, 2 * it:2 * it + 1],
                                     x16[:, TC * jc:TC * jc + TC],
                                     ones_col16[:], start=True, stop=True)
                    nc.tensor.matmul(st[:, jc, w, 2 * it + 1:2 * it + 2],
                                     sq[:, TC * jc:TC * jc + TC],
                                     ones_col16[:], start=True, stop=True)

        def derived(e):
            st = st_t.pop(e)
            tD, tDv = t_D[e % 2], t_Dv[e % 2]
            stS = sS.tile([TC, 2, 8, 6], F32, tag="stS")
            nc.vector.tensor_copy(stS[:], st[:])
            rr = stS[:].rearrange("p a w (b c) -> p a w b c", c=2)
            s1v, s2v = rr[:, :, :, :, 0], rr[:, :, :, :, 1]
            t1 = sS.tile([TC, 2, 8, 3], F32, tag="t1")
            nc.vector.tensor_tensor(t1[:], s1v, s1v, op=OP.mult)
            u0 = sS.tile([TC, 2, 8, 3], F32, tag="u0")
            nc.vector.tensor_scalar(u0[:], s2v, 128.0, 128.0 * 128.0 * EPS,
                                    op0=OP.mult, op1=OP.add)
            nc.vector.tensor_tensor(u0[:], u0[:], t1[:], op=OP.subtract)
            # rstd = 128/sqrt(u0): q -> D stat 0, k -> D stat 2, v -> Dv 0
            rstd = sS.tile([TC, 2, 8, 3], F32, tag="rstd")
            rsqrt_cols(u0[:], rstd[:], 128.0)
            Dr = tD[:].rearrange("p w (x y) a -> p w x y a", y=2)
            nc.vector.tensor_copy(
                Dr[:, :, :, 0, :],
                rstd[:, :, :, 0:2].rearrange("p a w c -> p w c a"))
            nc.vector.tensor_copy(tDv[:, :, :, 0:1], rstd[:, :, :, 2:3])
            # means scaled by 1/128: (mq, mk) -> tmp, mv -> Dv col 1
            m3 = sS.tile([TC, 2, 8, 3], F32, tag="m3")
            nc.vector.tensor_scalar(m3[:], s1v, 1.0 / 128.0, None, op0=OP.mult)
            nc.vector.tensor_copy(tDv[:, :, :, 1:2], m3[:, :, :, 2:3])
            # w = mean * rstd -> D stats {1, 3}
            nc.vector.tensor_tensor(
                Dr[:, :, :, 1, :],
                m3[:, :, :, 0:2].rearrange("p a w c -> p w c a"),
                rstd[:, :, :, 0:2].rearrange("p a w c -> p w c a"), op=OP.mult)
            if DEBUG and e == 0:
                nc.sync.dma_start(dbg["st"][:, :, :, :], stS[:])
                nc.sync.dma_start(dbg["rstd"][:, :, :, :], rstd[:])
                nc.sync.dma_start(dbg["D"][:, :, :, :], tD[:])

        def stage_B(W):
            e, w = W // 8, W % 8
            tD, tDv = t_D[e % 2], t_Dv[e % 2]
            t_q, t_k, t_v = slabs[e]
            if w == 7:
                del slabs[e]
            # stat row transpose: out row = 2*s + jc, q rows 0-3, k rows 4-7
            sp = p_scr.tile([8, TC], F16, tag="scr", name="sp")
            nc.tensor.transpose(sp[:, :], tD[:, w, :, :], ident16[:TC, :TC])
            spb = sS.tile([8, TC], F16, tag="spb")
            nc.vector.tensor_copy(spb[:], sp[:])
            s4 = sS.tile([1, 8 * TC], F16, tag="s4")
            nc.sync.dma_start(
                s4[0:1, :].rearrange("p (s t) -> p s t", s=8), spb[0:8, :])
            # broadcast: bcn[:, i, 0:216] = r, [:, i, 216:432] = w
            bcn = p_bcn.tile([C, 2, 512], F32, tag="bcn", name="bcn")
            for i in range(2):
                nc.tensor.matmul(bcn[:, i, 0:432], onesr16[:],
                                 s4[0:1, 432 * i:432 * i + 432],
                                 start=True, stop=True)
            # q^ / k^ (fp16, channel-major)
            q_win = t_q[:, w, :]
            k_win = t_k[:, w, :]
            tmq = tmpp.tile([C, T], F32, tag="tmq")
            nc.vector.tensor_tensor(tmq[:], q_win, bcn[:, 0, 0:T], op=OP.mult)
            qkn = qkp.tile([C, 2, T], F16, tag="qkn")
            nc.vector.tensor_tensor(qkn[:, 0, :], tmq[:], bcn[:, 0, T:2 * T],
                                    op=OP.subtract)
            if not trivial_q:
                nc.vector.tensor_scalar(qkn[:, 0, :], qkn[:, 0, :],
                                        t_gq[:, 0:1], t_bq[:, 0:1],
                                        op0=OP.mult, op1=OP.add)
            tmk = tmpp.tile([C, T], F32, tag="tmk")
            nc.vector.tensor_tensor(tmk[:], k_win, bcn[:, 1, 0:T], op=OP.mult)
            nc.vector.tensor_tensor(qkn[:, 1, :], tmk[:], bcn[:, 1, T:2 * T],
                                    op=OP.subtract)
            # restage head-major (q and k together): [16, 8, 2, 216]
            qkH = qkh.tile([DH, NH, 2, T], F16, tag="qkH", name="qkH")
            for h in range(NH):
                nc.sync.dma_start(qkH[:, h, :, :],
                                  qkn[DH * h:DH * h + DH, :, :])
            qkH_t[W] = qkH
            if DEBUG and W == 0:
                nc.sync.dma_start(dbg["qh"][:, :], qkn[:, 0, :])
                nc.sync.dma_start(dbg["kh"][:, :], qkn[:, 1, :])
                nc.sync.dma_start(dbg["qhH"][:, :, :], qkH[:, :, 0, :])
                nc.sync.dma_start(dbg["s4"][:, :], s4[:])
                nc.sync.dma_start(dbg["spb"][:, :], spb[:])
            # v^: fp16 cast, transpose to token-major (1 cyc/row + fp16
            # identity load), normalize per-partition
            v16 = tmpp.tile([C, T], F16, tag="v16")
            nc.scalar.copy(v16[:], t_v[:, w, :])
            vt = p_scr.tile([TC, 2, C], F16, tag="scr", name="vt")
            for jc in range(2):
                nc.tensor.transpose(vt[:, jc, :],
                                    v16[:, TC * jc:TC * jc + TC], ident16[:])
            tv = t_vh[W % NTV]
            for jc in range(2):
                vsrc = vt[:, jc, :].rearrange("p (r g d) -> p r g d", r=2, g=4)
                nc.vector.tensor_scalar(
                    tv[:, jc, :, :, 1:17], vsrc,
                    tDv[:, jc, w, 1:2], tDv[:, jc, w, 0:1],
                    op0=OP.subtract, op1=OP.mult)

        def scores_group(W, grp):
            jc, hh = grp // 2, grp % 2
            qkH = qkH_t[W]
            t_E = E_t[W]
            sc = p_big.tile([C, 4, 256], F32, tag="big", name="sc")
            for g in range(4):
                h = 4 * hh + g
                nc.tensor.matmul(sc[0:TC, g, 0:T],
                                 qkH[:, h, 1, TC * jc:TC * jc + TC],
                                 qkH[:, h, 0, :], start=True, stop=True)
            # exp in two halves: finer-grained PSUM release (subtile deps)
            # lets the next score group reuse the bank sooner
            nc.scalar.activation(t_E[:, jc, hh, 0:2, :], sc[0:TC, 0:2, 0:T],
                                 AF.Exp, scale=0.25)
            nc.scalar.activation(t_E[:, jc, hh, 2:4, :], sc[0:TC, 2:4, 0:T],
                                 AF.Exp, scale=0.25)

        def stage_D(W, part, state):
            t_E = E_t[W]
            tv = t_vh[W % NTV]
            if part == 0:
                av = p_av.tile([C, 2, T], F32, tag="av", name="av")
                state[W] = [av, None, None, None]
                if DEBUG and W == 0:
                    nc.sync.dma_start(dbg["tv"][:, :, :, :, :], tv[:])
                    nc.sync.dma_start(dbg["E"][:, :, :, :, :], t_E[:])
            av = state[W][0]
            if part in (0, 1):
                hh = part
                for g in range(4):
                    for jc in range(2):
                        nc.tensor.matmul(
                            av[32 * g:32 * g + 32, hh, :],
                            tv[:, jc, hh, g, :], t_E[:, jc, hh, g, :],
                            start=(jc == 0), stop=(jc == 1),
                            tile_position=(0, 32 * g))
            if part == 1:
                # 1/Z; issue ACT/DMA legs early so they clear the queues
                t_R = rp.tile([C, 2, T], F32, tag="R")
                nc.vector.reciprocal_approx_fast(
                    t_R[:].rearrange("p a b -> p (a b)"),
                    av[:].rearrange("p a b -> p (a b)"))
                t_Rh = rp.tile([C, 2, T], F16, tag="Rh")
                nc.scalar.copy(t_Rh[:], t_R[:])
                r4 = rp.tile([1, 4, 2, T], F16, tag="r4")
                for g in range(4):
                    nc.sync.dma_start(r4[0:1, g, :, :],
                                      t_Rh[32 * g:32 * g + 1, :, :])
                avS = avn_p.tile([C, 2, T], F16, tag="avS")
                nc.vector.tensor_copy(avS[:], av[:])
                state[W][1:4] = [t_R, r4, avS]
            if part == 2:
                av, t_R, r4, avS = state.pop(W)
                bp = p_bp.tile([C, 512], F32, tag="bp", name="bp")
                for g in range(4):
                    nc.tensor.matmul(
                        bp[32 * g:32 * g + 32, 0:2 * T],
                        onesr16[0:1, 0:32],
                        r4[0:1, g, :, :].rearrange("p a b -> p (a b)"),
                        start=True, stop=True, tile_position=(0, 32 * g))
                avn = avn_p.tile([C, 2, T], F16, tag="avn")
                bpv = bp[:, 0:2 * T].rearrange("p (a b) -> p a b", b=T)
                nc.vector.tensor_tensor(avn[:], avS[:], bpv, op=OP.mult)
                if DEBUG and W == 0:
                    nc.sync.dma_start(dbg["avn"][:, :, :], avn[:])
                    nc.sync.dma_start(dbg["R"][:, :, :], t_R[:])
                y = p_big.tile([C, 4, 256], F32, tag="big", name="y")
                nc.tensor.matmul(y[:, 0, 0:T], t_wt[0][:], avn[:, 0, :],
                                 start=True, stop=False)
                nc.tensor.matmul(y[:, 0, 0:T], t_wt[1][:], avn[:, 1, :],
                                 start=False, stop=True)
                e, w = W // 8, W % 8
                if w == 0:
                    oe_t[e] = outp.tile([C, 8, T], F32, tag="oe", name="oe")
                t_oe = oe_t[e]
                out_view = t_oe[:, w, :]
                if trivial_bias:
                    nc.scalar.copy(out_view, y[:, 0, 0:T])
                else:
                    nc.vector.tensor_scalar(out_view, y[:, 0, 0:T],
                                            t_pb[:, 0:1], None, op0=OP.add)
                if w == 7:
                    nc.sync.dma_start(ys[:, e, :, :], oe_t.pop(e)[:])

        dstate = {}
        for W in range(NW + 17):
            WA, WB, WC, WD = W, W - 8, W - 16, W - 17
            # emission order spreads the independent A/B matmuls between
            # the C score groups and ahead of D's Z-broadcast so the PE
            # queue always has ready work while exp / the 1/Z DMA land
            if 0 <= WC < NW:
                E_t[WC] = ep.tile([TC, 2, 2, 4, T], F16, tag="E", name="E")
                scores_group(WC, 0)
            if 0 <= WD < NW:
                stage_D(WD, 0, dstate)
            if 0 <= WC < NW:
                scores_group(WC, 1)
            if 0 <= WD < NW:
                stage_D(WD, 1, dstate)
            if 0 <= WC < NW:
                scores_group(WC, 2)
            if WA < NW:
                stage_A(WA)
            if 0 <= WC < NW:
                scores_group(WC, 3)
            if 0 <= WB < NW:
                stage_B(WB)
            if 0 <= WD < NW:
                stage_D(WD, 2, dstate)
            if WA < NW and WA % 8 == 7:
                derived(WA // 8)
            if 0 <= WC < NW - 1:
                qkH_t[WC - 1 if WC else 0] = None

    nc.compile()
    _BUILD_CACHE[key] = nc
    return nc


def _prepare(inputs):
    q_map = np.asarray(inputs["q_map"], np.float32)
    k_map = np.asarray(inputs["k_map"], np.float32)
    v_map = np.asarray(inputs["v_map"], np.float32)
    gamma_q = np.asarray(inputs["gamma_q"], np.float32)
    beta_q = np.asarray(inputs["beta_q"], np.float32)
    gamma_kv = np.asarray(inputs["gamma_kv"], np.float32)
    beta_kv = np.asarray(inputs["beta_kv"], np.float32)
    proj_w = np.asarray(inputs["proj_w"], np.float32)
    proj_b = np.asarray(inputs["proj_b"], np.float32)

    trivial_q = bool(np.all(gamma_q == 1.0) and np.all(beta_q == 0.0))
    trivial_kv = bool(np.all(gamma_kv == 1.0) and np.all(beta_kv == 0.0))
    if not trivial_kv:
        raise NotImplementedError(
            "nontrivial gamma_kv/beta_kv on k not implemented")

    # gamma_kv folds into the projection weight columns; beta_kv adds
    # proj_w @ beta_kv to every output (softmax rows sum to 1).
    wt_v = proj_w.T * gamma_kv[:, None]   # [c_in, c_out]
    bias = proj_b + proj_w @ beta_kv
    trivial_bias = bool(np.all(bias == 0.0))

    # packed av-row layout: row 32g+1+d (hh slot) = channel 16*(4hh+g)+d
    wt0 = np.zeros((C, C), np.float32)
    wt1 = np.zeros((C, C), np.float32)
    for g in range(4):
        for d in range(DH):
            wt0[32 * g + 1 + d] = wt_v[DH * g + d]
            wt1[32 * g + 1 + d] = wt_v[DH * (4 + g) + d]

    def to_windows(x, m):
        # [C, 6, 48, 48] -> [C, 8hw, 8ww, 216] with token order (d, h, w)
        s = x[0, :, 6 * m:6 * m + 6]
        s = s.reshape(C, 6, 8, 6, 8, 6)
        s = np.transpose(s, (0, 2, 4, 1, 3, 5))
        return np.ascontiguousarray(s.reshape(C, 8, 8, T))

    in_maps = []
    for m in range(NCORES):
        im = {
            "q_slab": to_windows(q_map, m),
            "k_slab": to_windows(k_map, m),
            "v_slab": to_windows(v_map, m),
            "wt0": wt0.astype(np.float16),
            "wt1": wt1.astype(np.float16),
            "pbias": np.ascontiguousarray(bias.reshape(C, 1)),
        }
        if not trivial_q:
            im["gq"] = np.ascontiguousarray(gamma_q.reshape(C, 1))
            im["bq"] = np.ascontiguousarray(beta_q.reshape(C, 1))
        in_maps.append(im)
    return (trivial_q, trivial_bias), in_maps


def _run(inputs, trace=False, **trace_kwargs):
    flags, in_maps = _prepare(inputs)
    nc = _build_nc(*flags)
    res = run_bass_kernel_spmd(nc, in_maps, list(range(NCORES)),
                               trace=trace, **trace_kwargs)
    slabs = []
    for m in range(NCORES):
        s = res.results[m]["y_slab"].reshape(C, 8, 8, 6, 6, 6)
        s = np.transpose(s, (0, 3, 1, 4, 2, 5)).reshape(C, 6, 48, 48)
        slabs.append(s)
    out = np.concatenate(slabs, axis=1).reshape(1, C, 48, 48, 48)
    return out.astype(np.float32), res


def kernel(**inputs):
    out, _ = _run(inputs, trace=False)
    return out


def kernel_traced(**inputs):
    return _run(inputs, trace=True)
